# revision 10
# baseline (speedup 1.0000x reference)
"""LSTM decoder (nn_Decoder) on 8 Trainium2 NeuronCores.

Strategy (v2):
  - Replicate the sequential LSTM recurrence on all 8 cores (serial in T;
    B=32 gives too little parallelism to shard), shard the output head over
    the vocab dim: core c computes logits[:, :, c*4000:(c+1)*4000].
    Unshard = host-side concat; no device collectives.
  - Input-side gate projections x_gates = e @ W_ih.T + b_ih + b_hh are
    computed on host (input-only prep, 5% of FLOPs) and injected into the
    PSUM accumulation base by a GpSimd (Pool engine) copy — removes the
    e-part matmuls AND the K=1 bias matmuls from the PE entirely.
  - Recurrence matmuls: stationary h^T [128,32] replicated into the PE's 4
    column groups (tile_position=(0,32j)) -> 4 weight streams concurrently,
    full 128-wide PE at M=32. start=False accumulates onto the copied xg.
  - h states kept in a rolling 4-step window tile (partition=hidden-in-chunk,
    free = k*128 + s*32 + b) that serves BOTH as next-step matmul stationary
    and as the head matmul stationary for token tile m = t//4.
  - The output head (8 vocab tiles of 500 per core, all W_out resident in
    SBUF) is interleaved into the recurrence: after each step, 2 head units
    (8 K-chunk matmuls each) fill the PE gap left by the elementwise tail.
  - All matmuls bf16 inputs / fp32 PSUM; gate elementwise in fp32.
"""

import numpy as np
import ml_dtypes
from contextlib import ExitStack

import concourse.bass as bass  # noqa: F401
import concourse.tile as tile
import concourse.bacc as bacc
import concourse.mybir as mybir
from concourse import bass_utils

BF16 = ml_dtypes.bfloat16
N_CORES = 8
B, T = 32, 128
VOCAB, D_EMB, Z_DIM, HID = 32000, 512, 256, 1024
VSH = VOCAB // N_CORES    # 4000 vocab per core
NTOK = B * T              # 4096 tokens; token index = t*32 + b
KH = HID // 128           # 8 hidden K-chunks
GW = 4 * HID              # 4096 gate width
NT_HEAD = 8               # vocab tiles per core in the head
NV = VSH // NT_HEAD       # 500
XPT = 8                   # steps per xg prefetch tile

_NC_CACHE = {}


def _perm():
    # psum gate order n = j*1024 + gate*256 + u  ->  torch W column gate*1024 + j*256 + u
    j = np.arange(4)[:, None, None]
    gate = np.arange(4)[None, :, None]
    u = np.arange(256)[None, None, :]
    return (gate * 1024 + j * 256 + u).reshape(-1)


def _build(repeat=1):
    if repeat in _NC_CACHE:
        return _NC_CACHE[repeat]
    nc = bacc.Bacc("TRN2", debug=False, num_devices=N_CORES)
    dt = mybir.dt
    xg_d = nc.dram_tensor("xg", [128, T * 1024], dt.bfloat16, kind="ExternalInput").ap()
    h0T_d = nc.dram_tensor("h0T", [128, KH * B], dt.bfloat16, kind="ExternalInput").ap()
    Ws_d = nc.dram_tensor("Ws", [128, KH * GW], dt.bfloat16, kind="ExternalInput").ap()
    id_d = nc.dram_tensor("ident", [128, 128], dt.bfloat16, kind="ExternalInput").ap()
    WoT_d = nc.dram_tensor("WoT", [128, NT_HEAD * KH * NV], dt.bfloat16, kind="ExternalInput").ap()
    biaso_d = nc.dram_tensor("bias_o", [128, VSH], dt.bfloat16, kind="ExternalInput").ap()
    out_d = nc.dram_tensor("out", [NTOK, VSH], dt.float32, kind="ExternalOutput").ap()

    with tile.TileContext(nc) as tc, ExitStack() as ctx:
        pers = ctx.enter_context(tc.tile_pool(name="pers", bufs=1))
        ident = pers.tile([128, 128], dt.bfloat16)
        nc.sync.dma_start(ident[:], id_d)
        for _rep in range(repeat):
            _emit_body(nc, tc, ident, xg_d, h0T_d, Ws_d, WoT_d, biaso_d, out_d)
    nc.compile()
    _NC_CACHE[repeat] = nc
    return nc


def _emit_body(nc, tc, ident, xg_d, h0T_d, Ws_d, WoT_d, biaso_d, out_d):
    dt = mybir.dt
    ACT = mybir.ActivationFunctionType
    with ExitStack() as rctx:
        wpool = rctx.enter_context(tc.tile_pool(name="ws", bufs=1))
        h0T_s = wpool.tile([128, KH * B], dt.bfloat16)
        nc.sync.dma_start(h0T_s[:], h0T_d)
        ws = wpool.tile([128, KH * GW], dt.bfloat16)        # 64KB/part
        for k in range(KH):
            nc.sync.dma_start(ws[:, k * GW:(k + 1) * GW], Ws_d[:, k * GW:(k + 1) * GW])
        wo = wpool.tile([128, NT_HEAD * KH * NV], dt.bfloat16)  # 64KB/part
        nc.sync.dma_start(wo[:], WoT_d)
        bias_o = wpool.tile([128, VSH], dt.bfloat16)
        nc.sync.dma_start(bias_o[:], biaso_d)

        xgpool = rctx.enter_context(tc.tile_pool(name="xg", bufs=2))
        hwin = rctx.enter_context(tc.tile_pool(name="hw", bufs=3))
        gpsum = rctx.enter_context(tc.tile_pool(name="gps", bufs=2, space="PSUM"))
        tpsum = rctx.enter_context(tc.tile_pool(name="tps", bufs=1, space="PSUM"))
        hpsum = rctx.enter_context(tc.tile_pool(name="hps", bufs=3, space="PSUM"))
        ew = rctx.enter_context(tc.tile_pool(name="ew", bufs=2))
        opool = rctx.enter_context(tc.tile_pool(name="osb", bufs=6))
        cpool = rctx.enter_context(tc.tile_pool(name="cst", bufs=1))

        c_sb = cpool.tile([128, 256], dt.float32)
        nc.vector.memset(c_sb[:], 0.0)

        def mm_block(g, lhsT, k, nh, start, stop):
            for j in range(4):
                nc.tensor.matmul(
                    g[32 * j:32 * j + 32, nh * 512:(nh + 1) * 512],
                    lhsT,
                    ws[:, k * GW + j * 1024 + nh * 512:
                       k * GW + j * 1024 + (nh + 1) * 512],
                    start=start, stop=stop,
                    tile_position=(0, 32 * j),
                    skip_group_check=True,
                )

        head_q = []

        def emit_head_unit():
            if not head_q:
                return
            hw_m, m, nt = head_q.pop(0)
            ps = hpsum.tile([128, NV], dt.float32, tag="hp")
            for k in range(KH):
                nc.tensor.matmul(
                    ps[:],
                    hw_m[:, k * 128:(k + 1) * 128],
                    wo[:, (nt * KH + k) * NV:(nt * KH + k + 1) * NV],
                    start=(k == 0), stop=(k == KH - 1),
                )
            osb = opool.tile([128, NV], dt.float32, tag="osb")
            nc.vector.tensor_add(osb[:], ps[:], bias_o[:, nt * NV:(nt + 1) * NV])
            nc.sync.dma_start(
                out_d[m * 128:(m + 1) * 128, nt * NV:(nt + 1) * NV], osb[:])

        xg_cur = xgpool.tile([128, XPT * 1024], dt.bfloat16, tag="xg")
        nc.sync.dma_start(xg_cur[:], xg_d[:, 0:XPT * 1024])
        xg_next = xgpool.tile([128, XPT * 1024], dt.bfloat16, tag="xg")
        nc.sync.dma_start(xg_next[:], xg_d[:, XPT * 1024:2 * XPT * 1024])
        hw_prev = None
        hw = None
        for t in range(T):
            if t % XPT == 0 and t > 0:
                xg_cur = xg_next
                if t + XPT < T:
                    xg_next = xgpool.tile([128, XPT * 1024], dt.bfloat16, tag="xg")
                    nc.sync.dma_start(
                        xg_next[:], xg_d[:, (t + XPT) * 1024:(t + 2 * XPT) * 1024])
            xg_t = xg_cur[:, (t % XPT) * 1024:(t % XPT + 1) * 1024]
            if t % 4 == 0:
                hw = hwin.tile([128, 1024], dt.bfloat16, tag="hw")
            g = gpsum.tile([128, 1024], dt.float32, tag="g")
            # First touch of each psum buffer: accumulation-zone state is
            # inherited from the previous NEFF, so start=True and add xg on
            # the DVE instead. Steady state: xg copied into psum by the
            # scalar engine, matmuls accumulate on top (start=False).
            prime = t < 2
            if not prime:
                nc.scalar.activation(g[:], xg_t, ACT.Copy)
            for nh in range(2):
                for k in range(KH):
                    if t == 0:
                        lhsT = h0T_s[:, k * 32:(k + 1) * 32]
                    else:
                        off = k * 128 + ((t - 1) % 4) * 32
                        lhsT = hw_prev[:, off:off + 32]
                    mm_block(g, lhsT, k, nh, prime and k == 0, k == KH - 1)
            if prime:
                gx = ew.tile([128, 1024], dt.float32, tag="gx")
                nc.vector.tensor_add(gx[:], g[:], xg_t)
                gsrc = gx
            else:
                gsrc = g

            emit_head_unit()

            if_sb = ew.tile([128, 512], dt.float32, tag="if")
            nc.scalar.activation(if_sb[:], gsrc[:, 0:512], ACT.Sigmoid)
            gg_sb = ew.tile([128, 256], dt.float32, tag="gg")
            nc.scalar.activation(gg_sb[:], gsrc[:, 512:768], ACT.Tanh)
            o_sb = ew.tile([128, 256], dt.float32, tag="o")
            nc.scalar.activation(o_sb[:], gsrc[:, 768:1024], ACT.Sigmoid)
            nc.vector.tensor_mul(c_sb[:], c_sb[:], if_sb[:, 256:512])
            t1 = ew.tile([128, 256], dt.float32, tag="t1")
            nc.vector.tensor_mul(t1[:], if_sb[:, 0:256], gg_sb[:])
            nc.vector.tensor_add(c_sb[:], c_sb[:], t1[:])
            tc_sb = ew.tile([128, 256], dt.float32, tag="tc")
            nc.scalar.activation(tc_sb[:], c_sb[:], ACT.Tanh)
            h_bf = ew.tile([128, 256], dt.bfloat16, tag="h")
            nc.vector.tensor_mul(h_bf[:], o_sb[:], tc_sb[:])
            hw_v = hw[:].rearrange("p (j mh s b) -> p j mh s b", j=4, mh=2, s=4)
            # Both 128-wide transposes share one psum bank: the first arms the
            # 2KB zero-region (start=True), the second lands on still-pending
            # bytes (start=False -> zero-then-write), then one combined copy.
            tr = tpsum.tile([128, 256], dt.bfloat16, tag="tr")
            for mh in range(2):
                nc.tensor.matmul(
                    tr[:, mh * 128:(mh + 1) * 128],
                    h_bf[:, mh * 128:(mh + 1) * 128], ident[:],
                    is_transpose=True, start=(mh == 0), stop=(mh == 1),
                    skip_group_check=True,
                )
            nc.vector.tensor_copy(
                hw_v[:, :, :, t % 4, :],
                tr[:].rearrange("p (mh j b) -> p j mh b", mh=2, j=4),
            )
            hw_prev = hw

            emit_head_unit()

            if t % 4 == 3:
                m = t // 4
                for nt in range(NT_HEAD):
                    head_q.append((hw, m, nt))

        while head_q:
            emit_head_unit()


def prep_in_maps(z, x, W_h, b_h, emb, W_ih, W_hh, b_ih, b_hh, W_out, b_out):
    f32 = np.float32
    z = np.asarray(z, f32)
    W_h = np.asarray(W_h, f32)
    b_h = np.asarray(b_h, f32)
    emb = np.asarray(emb, f32)
    W_ih = np.asarray(W_ih, f32)
    W_hh = np.asarray(W_hh, f32)
    b_ih = np.asarray(b_ih, f32)
    b_hh = np.asarray(b_hh, f32)
    W_out = np.asarray(W_out, f32)
    b_out = np.asarray(b_out, f32)
    x = np.asarray(x)

    h0 = np.tanh(z @ W_h.T + b_h)                       # [B, H]
    e = emb[x]                                          # [B, T, D]
    # x_gates on host: [B, T, 4H] in torch col order gate*1024 + hidden
    xg_full = e.reshape(-1, D_EMB) @ W_ih.T + (b_ih + b_hh)
    # psum layout: xg[t, p=32j+b, col=g*256+u] = xg_full[b, t, g*1024 + j*256 + u]
    xg_arr = xg_full.reshape(B, T, 4, 4, 256).transpose(1, 3, 0, 2, 4)  # [T,j,b,g,u]
    xg_arr = xg_arr.reshape(T, 128, 1024)
    xg = np.ascontiguousarray(xg_arr.transpose(1, 0, 2)).reshape(128, T * 1024)
    # h0T[p, k*32+b] = h0[b, k*128+p]
    h0T = np.ascontiguousarray(h0.T.reshape(KH, 128, B).transpose(1, 0, 2)).reshape(128, KH * B)
    perm = _perm()
    Wp = W_hh.T[:, perm]                                # [H, 4H]
    Ws = np.ascontiguousarray(Wp.reshape(KH, 128, GW).transpose(1, 0, 2)).reshape(128, KH * GW)
    ident = np.eye(128, dtype=BF16)

    base = {
        "xg": xg.astype(BF16),
        "h0T": h0T.astype(BF16),
        "Ws": Ws.astype(BF16),
        "ident": ident,
    }
    in_maps = []
    for c in range(N_CORES):
        Wsh = W_out[c * VSH:(c + 1) * VSH]              # [4000, 1024]
        WoT = np.ascontiguousarray(
            Wsh.reshape(NT_HEAD, NV, KH, 128).transpose(3, 0, 2, 1)
        ).reshape(128, NT_HEAD * KH * NV)
        bsh = b_out[c * VSH:(c + 1) * VSH]
        bias_o = np.ascontiguousarray(np.broadcast_to(bsh, (128, VSH)))
        m = dict(base)
        m["WoT"] = WoT.astype(BF16)
        m["bias_o"] = bias_o.astype(BF16)
        in_maps.append(m)
    return in_maps


def assemble(results):
    outs = [np.asarray(r["out"]).reshape(T, B, VSH) for r in results]
    full = np.concatenate(outs, axis=2)                 # [T, B, VOCAB]
    return np.ascontiguousarray(full.transpose(1, 0, 2))


def kernel(**inputs):
    in_maps = prep_in_maps(**inputs)
    nc = _build()
    res = bass_utils.run_bass_kernel_spmd(nc, in_maps, core_ids=list(range(N_CORES)))
    return assemble(res.results)
